# revision 8
# baseline (speedup 1.0000x reference)
"""Bass/Tile TRN2 kernel for an (intentionally quirky) nn.MultiHeadAttention.

Problem shapes: B=8, S=256, D=4096, H=16, HD=256.
Sharding: pure data-parallel - one batch element per NeuronCore (8 cores).

Math (per batch b, with m[j] = float(mask[b, j] != 0)):
    Q = (x_q @ Wq.T + bq) / 16           (1/sqrt(HD) folded into Q)
    K = (x_k @ Wk.T + bk) * m[c mod 256] (the module masks head-dim channels;
                                          masking K only is exact since m^2 = m)
    V = x_v @ Wv.T + bv
    per head h: S_T[t, s] = sum_hd K_h[t,hd] Q_h[s,hd]   (scores, transposed)
                P = exp(S_T)              (no max-sub needed; |scores| <~ 6)
                r[s] = 1 / sum_t P[t, s]  (via ones-matmul over partitions)
                A_h[hd, s] = (sum_t V_h[t,hd] P[t,s]) * r[s]
    out_T = (Wo/64) @ concat_h(A_h) + bo  ->  host transposes back.

All matmuls run in bf16 (1 cycle/row on PE) with fp32 PSUM accumulation.
"""

import sys
import types

sys.path.insert(0, "/opt/trn_rl_repo")

import numpy as np
import ml_dtypes

import concourse.bass as bass
import concourse.mybir as mybir
import concourse.tile as tile
from concourse.vector_clock import ScopedClock

BF16 = mybir.dt.bfloat16
F32 = mybir.dt.float32
NPBF16 = ml_dtypes.bfloat16

B, S, D, H = 8, 256, 4096, 16
HD = D // H          # 256
NK = D // 128        # 32 k-tiles of 128
NC = D // 128        # 32 output-channel tiles of 128
NG = 4               # dout groups of 8 tiles (8 PSUM banks)
N_CORES = 8

_drain_patched = False


def _patch_tile_drain():
    """This container's walrus build accepts only one sync-wait per
    instruction; Tile's exit drain collects one wait per logical processor.
    Split the waits across a chain of drains."""
    global _drain_patched
    if _drain_patched:
        return
    _drain_patched = True

    def patched(self, tick_clock, wait_clock):
        drain_inst = self.nc.sync.drain()
        inst = drain_inst.ins
        wait_clock.add_sem_waits(inst, ScopedClock({None: tick_clock.global_clock}))
        si = inst.sync_info
        if si is not None and len(si.on_wait) > 1:
            waits = list(si.on_wait)
            inst.sync_info = mybir.SyncInfo(
                on_wait=waits[:1], on_update=list(si.on_update)
            )
            for i in range(1, len(waits)):
                extra = self.nc.sync.drain()
                extra.ins.sync_info = mybir.SyncInfo(
                    on_wait=waits[i : i + 1], on_update=[]
                )
        self.nc.all_engine_barrier()
        popped = self.nc._tile_sem_poison_stack.pop()
        assert popped is self._sem_poison
        self.nc.clear_and_free_semaphores(list(self.sems.allocated().values()))
        self.nc.all_engine_barrier()

    tile.TileContext._drain_and_barrier = patched


_bir_patched = False


def _patch_bir_wait_split():
    """This walrus build accepts a single sync-wait per instruction. Tile's
    wait-assignment emits up to 4. Split them in the serialized BIR: extra
    waits are carried by no-op RegisterMove instructions (imm 0 ->
    {Engine}_zero) inserted just before the overloaded instruction on the
    same engine. Monotonic sem-ge waits make sequential waiting equivalent
    to simultaneous waiting."""
    global _bir_patched
    if _bir_patched:
        return
    _bir_patched = True
    import json as _json

    import concourse.bass2jax as b2j

    orig_compile = b2j.compile_bir_kernel

    def split_waits(bir_str):
        m = _json.loads(bir_str)
        changed = False
        for fn in m.get("functions", []):
            for blk in fn.get("blocks", []):
                insts = blk.get("instructions", [])
                out = []
                for inst in insts:
                    si = inst.get("sync_info") or {}
                    waits = si.get("on_wait") or []
                    if len(waits) > 1 and all(
                        w.get("wait_mode") == "sem-ge-imm" for w in waits
                    ):
                        changed = True
                        eng = inst["engine"]
                        for i, w in enumerate(waits[:-1]):
                            out.append(
                                {
                                    "debug": inst.get("debug", 0),
                                    "engine": eng,
                                    "ins": [
                                        {
                                            "dtype": "int32",
                                            "kind": "imm_value",
                                            "value": 0,
                                        }
                                    ],
                                    "name": f"{inst['name']}_w{i}",
                                    "opcode": "RegisterMove",
                                    "outs": [
                                        {
                                            "dtype": "int32",
                                            "kind": "register_access",
                                            "regref": f"{eng}_zero",
                                        }
                                    ],
                                    "sync_info": {
                                        "on_update": [],
                                        "on_wait": [w],
                                    },
                                }
                            )
                        inst["sync_info"] = {
                            "on_update": si.get("on_update") or [],
                            "on_wait": [waits[-1]],
                        }
                    out.append(inst)
                blk["instructions"] = out
        if not changed:
            return bir_str
        return _json.dumps(m).encode()

    def wrapped(ant_bir_str, *args, **kwargs):
        return orig_compile(split_waits(ant_bir_str), *args, **kwargs)

    b2j.compile_bir_kernel = wrapped


def _install_ntff_hook():
    """Recreate the missing antenv.axon_hooks glue so trace=True can profile."""
    if "antenv.axon_hooks" in sys.modules:
        return
    mod = types.ModuleType("antenv.axon_hooks")
    mod._hook = None
    mod.set_axon_ntff_profile_hook = lambda h: setattr(mod, "_hook", h)
    mod.get_axon_ntff_profile_hook = lambda: mod._hook
    sys.modules["antenv.axon_hooks"] = mod
    try:
        import antenv

        antenv.axon_hooks = mod
        if "/root/.axon_site" not in sys.path:
            sys.path.insert(0, "/root/.axon_site")
        from trn_agent_boot.trn_boot import _ntff_profile_via_ctypes

        mod._hook = _ntff_profile_via_ctypes("/opt/axon/libaxon_pjrt.so")
        import concourse.bass_utils as bu

        bu.upload_artifacts = lambda tmpdir: tmpdir
    except Exception:
        pass


def build_nc():
    _patch_tile_drain()
    nc = bass.Bass()

    xq = nc.dram_tensor("xq", [NK, 128, S], BF16, kind="ExternalInput")
    xk = nc.dram_tensor("xk", [NK, 128, S], BF16, kind="ExternalInput")
    xv = nc.dram_tensor("xv", [NK, 128, S], BF16, kind="ExternalInput")
    wq = nc.dram_tensor("wq", [NG, NK, 128, 1024], BF16, kind="ExternalInput")
    wk = nc.dram_tensor("wk", [NG, NK, 128, 1024], BF16, kind="ExternalInput")
    wo = nc.dram_tensor("wo", [NG, NK, 128, 1024], BF16, kind="ExternalInput")
    wv = nc.dram_tensor("wv", [8, NK, 128, 512], BF16, kind="ExternalInput")
    bqv = nc.dram_tensor("bqv", [128, NC], F32, kind="ExternalInput")
    bkv = nc.dram_tensor("bkv", [128, NC], F32, kind="ExternalInput")
    kmv = nc.dram_tensor("kmv", [128, NC], F32, kind="ExternalInput")
    bov = nc.dram_tensor("bov", [128, NC], F32, kind="ExternalInput")
    bvv = nc.dram_tensor("bvv", [1, D], BF16, kind="ExternalInput")
    out = nc.dram_tensor("out", [NC, 128, S], F32, kind="ExternalOutput")

    Ident = mybir.ActivationFunctionType.Identity
    Exp = mybir.ActivationFunctionType.Exp

    with tile.TileContext(nc) as tc:
        from contextlib import ExitStack

        with ExitStack() as ctx:
            resid = ctx.enter_context(tc.tile_pool(name="resid", bufs=1))
            wpool = ctx.enter_context(tc.tile_pool(name="wch", bufs=12))
            attnp = ctx.enter_context(tc.tile_pool(name="attnp", bufs=4))
            outp = ctx.enter_context(tc.tile_pool(name="outp", bufs=4))
            drp = ctx.enter_context(tc.tile_pool(name="drp", bufs=4, space="DRAM"))

            # ---- resident SBUF tensors ----
            xq_sb = resid.tile([128, NK, S], BF16, tag="xq")
            xk_sb = resid.tile([128, NK, S], BF16, tag="xk")
            xv_sb = resid.tile([128, NK, S], BF16, tag="xv")
            qt_sb = resid.tile([128, NC, S], BF16, tag="qt")  # Q^T  [dout, s]
            kt_sb = resid.tile([128, NC, S], BF16, tag="kt")  # K^T  [dout, t]
            v0_sb = resid.tile([128, D], BF16, tag="v0")      # V[t=0:128, c]
            v1_sb = resid.tile([128, D], BF16, tag="v1")      # V[t=128:256, c]
            at_sb = resid.tile([128, NC, S], BF16, tag="at")  # attn^T [c, s]
            bq_sb = resid.tile([128, NC], F32, tag="bq")
            bk_sb = resid.tile([128, NC], F32, tag="bk")
            km_sb = resid.tile([128, NC], F32, tag="km")
            bo_sb = resid.tile([128, NC], F32, tag="bo")
            bv_sb = resid.tile([1, D], BF16, tag="bv")
            ones1 = resid.tile([1, 128], BF16, tag="ones1")
            ones128 = resid.tile([128, 1], BF16, tag="ones128")

            nc.vector.memset(ones1[:], 1.0)
            nc.vector.memset(ones128[:], 1.0)
            nc.gpsimd.dma_start(out=bq_sb[:], in_=bqv[:])
            nc.gpsimd.dma_start(out=bk_sb[:], in_=bkv[:])
            nc.gpsimd.dma_start(out=km_sb[:], in_=kmv[:])
            nc.gpsimd.dma_start(out=bo_sb[:], in_=bov[:])
            nc.gpsimd.dma_start(out=bv_sb[:], in_=bvv[:])
            for kt in range(NK):
                nc.sync.dma_start(out=xq_sb[:, kt, :], in_=xq[kt])
                nc.sync.dma_start(out=xk_sb[:, kt, :], in_=xk[kt])
                nc.sync.dma_start(out=xv_sb[:, kt, :], in_=xv[kt])

            # ---- Q and K projections (weights stationary, output transposed) ----
            def proj_t(w_dram, x_sb, dst_sb, bias_sb, scale_sb, pspool):
                for g in range(NG):
                    ps = [
                        pspool.tile([128, S], F32, name="ps", tag="ps")
                        for _ in range(8)
                    ]
                    for kt in range(NK):
                        ch = wpool.tile([128, 1024], BF16, tag="ch")
                        nc.sync.dma_start(out=ch[:], in_=w_dram[g, kt])
                        for j in range(8):
                            nc.tensor.matmul(
                                ps[j][:],
                                lhsT=ch[:, j * 128 : (j + 1) * 128],
                                rhs=x_sb[:, kt, :],
                                start=(kt == 0),
                                stop=(kt == NK - 1),
                            )
                    for j in range(8):
                        c = g * 8 + j
                        scale = scale_sb[:, c : c + 1] if scale_sb is not None else 1.0
                        nc.scalar.activation(
                            out=dst_sb[:, c, :],
                            in_=ps[j][:],
                            func=Ident,
                            bias=bias_sb[:, c : c + 1],
                            scale=scale,
                        )

            with tc.tile_pool(name="psqk", bufs=8, space="PSUM") as psqk:
                proj_t(wq, xq_sb, qt_sb, bq_sb, None, psqk)
                proj_t(wk, xk_sb, kt_sb, bk_sb, km_sb, psqk)

            # ---- V projection (x stationary, natural layout [t, c]) ----
            with tc.tile_pool(name="psv", bufs=4, space="PSUM") as psv:
                for d in range(8):
                    pv0 = psv.tile([128, 512], F32, tag="pv")
                    pv1 = psv.tile([128, 512], F32, tag="pv")
                    sl = slice(d * 512, (d + 1) * 512)
                    # init PSUM with broadcast bias via K=1 matmul
                    nc.tensor.matmul(
                        pv0[:], lhsT=ones1[:], rhs=bv_sb[:, sl], start=True, stop=False
                    )
                    nc.tensor.matmul(
                        pv1[:], lhsT=ones1[:], rhs=bv_sb[:, sl], start=True, stop=False
                    )
                    for kt in range(NK):
                        ch = wpool.tile([128, 512], BF16, tag="ch")
                        nc.sync.dma_start(out=ch[:], in_=wv[d, kt])
                        last = kt == NK - 1
                        nc.tensor.matmul(
                            pv0[:],
                            lhsT=xv_sb[:, kt, 0:128],
                            rhs=ch[:],
                            start=False,
                            stop=last,
                        )
                        nc.tensor.matmul(
                            pv1[:],
                            lhsT=xv_sb[:, kt, 128:256],
                            rhs=ch[:],
                            start=False,
                            stop=last,
                        )
                    nc.scalar.activation(out=v0_sb[:, sl], in_=pv0[:], func=Ident)
                    nc.scalar.activation(out=v1_sb[:, sl], in_=pv1[:], func=Ident)

            # ---- attention, one head at a time ----
            with (
                tc.tile_pool(name="psa", bufs=6, space="PSUM") as psa,
                tc.tile_pool(name="psr", bufs=2, space="PSUM") as psr,
            ):
                for h in range(H):
                    c0 = 2 * h
                    exp_t = []
                    for tt in range(2):
                        pss = psa.tile([128, S], F32, tag="pa")
                        tsl = slice(tt * 128, (tt + 1) * 128)
                        for j in range(2):
                            nc.tensor.matmul(
                                pss[:],
                                lhsT=kt_sb[:, c0 + j, tsl],
                                rhs=qt_sb[:, c0 + j, :],
                                start=(j == 0),
                                stop=(j == 1),
                            )
                        et = attnp.tile([128, S], BF16, tag="exp")
                        nc.scalar.activation(out=et[:], in_=pss[:], func=Exp)
                        exp_t.append(et)
                    # column sums of exp (sum over t = partition dim) via matmul
                    prs = psr.tile([1, S], F32, tag="pr")
                    nc.tensor.matmul(
                        prs[:], lhsT=ones128[:], rhs=exp_t[0][:], start=True, stop=False
                    )
                    nc.tensor.matmul(
                        prs[:], lhsT=ones128[:], rhs=exp_t[1][:], start=False, stop=True
                    )
                    rsum = attnp.tile([1, S], F32, tag="rs")
                    nc.vector.reciprocal(rsum[:], prs[:])
                    # broadcast partition 0 to all 128 partitions by bouncing
                    # through DRAM (DMA from DRAM may replicate, SBUF may not)
                    rdr = drp.tile([1, S], F32, name="rdr", tag="rdr")
                    nc.gpsimd.dma_start(out=rdr[:], in_=rsum[:])
                    rbc = attnp.tile([128, S], F32, tag="rb")
                    rdr_b = bass.AP(
                        tensor=rdr.tensor,
                        offset=rdr.offset,
                        ap=[[0, 128], [1, S]],
                    )
                    nc.gpsimd.dma_start(out=rbc[:], in_=rdr_b)
                    for j in range(2):
                        csl = slice(h * HD + j * 128, h * HD + (j + 1) * 128)
                        pu = psa.tile([128, S], F32, tag="pa")
                        nc.tensor.matmul(
                            pu[:], lhsT=v0_sb[:, csl], rhs=exp_t[0][:],
                            start=True, stop=False,
                        )
                        nc.tensor.matmul(
                            pu[:], lhsT=v1_sb[:, csl], rhs=exp_t[1][:],
                            start=False, stop=True,
                        )
                        nc.vector.tensor_mul(at_sb[:, c0 + j, :], pu[:], rbc[:])

            # ---- output projection ----
            with tc.tile_pool(name="pso", bufs=8, space="PSUM") as pso:
                for g in range(NG):
                    ps = [
                        pso.tile([128, S], F32, name="po", tag="po") for _ in range(8)
                    ]
                    for ct in range(NC):
                        ch = wpool.tile([128, 1024], BF16, tag="ch")
                        nc.sync.dma_start(out=ch[:], in_=wo[g, ct])
                        for j in range(8):
                            nc.tensor.matmul(
                                ps[j][:],
                                lhsT=ch[:, j * 128 : (j + 1) * 128],
                                rhs=at_sb[:, ct, :],
                                start=(ct == 0),
                                stop=(ct == NC - 1),
                            )
                    for j in range(8):
                        c = g * 8 + j
                        ot = outp.tile([128, S], F32, tag="ot")
                        nc.scalar.activation(
                            out=ot[:],
                            in_=ps[j][:],
                            func=Ident,
                            bias=bo_sb[:, c : c + 1],
                        )
                        nc.sync.dma_start(out=out[c], in_=ot[:])

    return nc


_cached = {}


def _get_nc():
    if "nc" not in _cached:
        _cached["nc"] = build_nc()
    return _cached["nc"]


def _prep_shared(Wq, bq, Wk, bk, Wv, bv, Wo, bo):
    """Host-side weight reorganization (shared across cores)."""
    def chunks1024(W, scale):
        # W.T with dout split into 4 groups of 1024: [4, 32, 128, 1024]
        wt = (W.T * scale).astype(NPBF16)  # [k, dout]
        return np.ascontiguousarray(
            wt.reshape(NK, 128, NG, 1024).transpose(2, 0, 1, 3)
        )

    wq_c = chunks1024(Wq, 1.0 / 16.0)
    wk_c = chunks1024(Wk, 1.0)
    wo_c = chunks1024(Wo, 1.0 / 64.0)
    # Wv.T with dout split into 8 chunks of 512: [8, 32, 128, 512]
    wv_t = Wv.T.astype(NPBF16)
    wv_c = np.ascontiguousarray(wv_t.reshape(NK, 128, 8, 512).transpose(2, 0, 1, 3))

    bqv = np.ascontiguousarray((bq / 16.0).astype(np.float32).reshape(NC, 128).T)
    bov = np.ascontiguousarray(bo.astype(np.float32).reshape(NC, 128).T)
    bvv = np.ascontiguousarray(bv.astype(NPBF16).reshape(1, D))
    return wq_c, wk_c, wv_c, wo_c, bqv, bov, bvv


def kernel(q, k, v, mask, Wq, bq, Wk, bk, Wv, bv, Wo, bo):
    q = np.asarray(q, dtype=np.float32)
    k = np.asarray(k, dtype=np.float32)
    v = np.asarray(v, dtype=np.float32)
    mask = np.asarray(mask)
    Wq, bq = np.asarray(Wq, np.float32), np.asarray(bq, np.float32)
    Wk, bk = np.asarray(Wk, np.float32), np.asarray(bk, np.float32)
    Wv, bv = np.asarray(Wv, np.float32), np.asarray(bv, np.float32)
    Wo, bo = np.asarray(Wo, np.float32), np.asarray(bo, np.float32)

    _install_ntff_hook()
    _patch_bir_wait_split()
    nc = _get_nc()
    wq_c, wk_c, wv_c, wo_c, bqv, bov, bvv = _prep_shared(
        Wq, bq, Wk, bk, Wv, bv, Wo, bo
    )

    in_maps = []
    for b in range(B):
        m = (mask[b] != 0).astype(np.float32)  # [256]
        mfull = np.tile(m, H)                  # [4096] mask per channel
        bkv = np.ascontiguousarray((bk * mfull).reshape(NC, 128).T.astype(np.float32))
        kmv = np.ascontiguousarray(mfull.reshape(NC, 128).T.astype(np.float32))

        def xt(x):
            return np.ascontiguousarray(x[b].T).astype(NPBF16).reshape(NK, 128, S)

        in_maps.append(
            dict(
                xq=xt(q), xk=xt(k), xv=xt(v),
                wq=wq_c, wk=wk_c, wv=wv_c, wo=wo_c,
                bqv=bqv, bkv=bkv, kmv=kmv, bov=bov, bvv=bvv,
            )
        )

    from concourse.bass_utils import run_bass_kernel_spmd

    res = run_bass_kernel_spmd(nc, in_maps, list(range(N_CORES)))
    outs = []
    for b in range(B):
        ot = res.results[b]["out"]  # [32, 128, 256]
        outs.append(ot.reshape(D, S).T)  # [256, 4096]
    return np.ascontiguousarray(np.stack(outs)).astype(np.float32)


# revision 15
# speedup vs baseline: 1.0611x; 1.0611x over previous
"""Bass/Tile TRN2 kernel for an (intentionally quirky) nn.MultiHeadAttention.

Problem shapes: B=8, S=256, D=4096, H=16, HD=256.
Sharding: pure data-parallel - one batch element per NeuronCore (8 cores).

Math (per batch b, with m[j] = float(mask[b, j] != 0)):
    Q = (x_q @ Wq.T + bq) / 16           (1/sqrt(HD) folded into Q)
    K = (x_k @ Wk.T + bk) * m[c mod 256] (the module masks head-dim channels;
                                          masking K only is exact since m^2 = m)
    V = x_v @ Wv.T + bv
    per head h: S_T[t, s] = sum_hd K_h[t,hd] Q_h[s,hd]   (scores, transposed)
                P = exp(S_T)              (no max-sub needed; |scores| <~ 6)
                r[s] = 1 / sum_t P[t, s]  (via ones-matmul over partitions)
                A_h[hd, s] = (sum_t V_h[t,hd] P[t,s]) * r[s]
    out_T = (Wo/64) @ concat_h(A_h) + bo  ->  host transposes back.

All matmuls run in bf16 (1 cycle/row on PE) with fp32 PSUM accumulation.
"""

import sys
import types

sys.path.insert(0, "/opt/trn_rl_repo")

import numpy as np
import ml_dtypes

import concourse.bass as bass
import concourse.mybir as mybir
import concourse.tile as tile
from concourse.vector_clock import ScopedClock

BF16 = mybir.dt.bfloat16
F32 = mybir.dt.float32
NPBF16 = ml_dtypes.bfloat16

B, S, D, H = 8, 256, 4096, 16
HD = D // H          # 256
NK = D // 128        # 32 k-tiles of 128
NC = D // 128        # 32 output-channel tiles of 128
NG = 4               # dout groups of 8 tiles (8 PSUM banks)
N_CORES = 8

_drain_patched = False


def _patch_tile_drain():
    """This container's walrus build accepts only one sync-wait per
    instruction; Tile's exit drain collects one wait per logical processor.
    Split the waits across a chain of drains."""
    global _drain_patched
    if _drain_patched:
        return
    _drain_patched = True

    def patched(self, tick_clock, wait_clock):
        drain_inst = self.nc.sync.drain()
        inst = drain_inst.ins
        wait_clock.add_sem_waits(inst, ScopedClock({None: tick_clock.global_clock}))
        si = inst.sync_info
        if si is not None and len(si.on_wait) > 1:
            waits = list(si.on_wait)
            inst.sync_info = mybir.SyncInfo(
                on_wait=waits[:1], on_update=list(si.on_update)
            )
            for i in range(1, len(waits)):
                extra = self.nc.sync.drain()
                extra.ins.sync_info = mybir.SyncInfo(
                    on_wait=waits[i : i + 1], on_update=[]
                )
        self.nc.all_engine_barrier()
        popped = self.nc._tile_sem_poison_stack.pop()
        assert popped is self._sem_poison
        self.nc.clear_and_free_semaphores(list(self.sems.allocated().values()))
        self.nc.all_engine_barrier()

    tile.TileContext._drain_and_barrier = patched


_bir_patched = False


def _patch_bir_wait_split():
    """This walrus build accepts a single sync-wait per instruction. Tile's
    wait-assignment emits up to 4. Split them in the serialized BIR: extra
    waits are carried by no-op RegisterMove instructions (imm 0 ->
    {Engine}_zero) inserted just before the overloaded instruction on the
    same engine. Monotonic sem-ge waits make sequential waiting equivalent
    to simultaneous waiting."""
    global _bir_patched
    if _bir_patched:
        return
    _bir_patched = True
    import json as _json

    import concourse.bass2jax as b2j

    orig_compile = b2j.compile_bir_kernel

    def split_waits(bir_str):
        m = _json.loads(bir_str)
        changed = False
        for fn in m.get("functions", []):
            for blk in fn.get("blocks", []):
                insts = blk.get("instructions", [])
                out = []
                for inst in insts:
                    si = inst.get("sync_info") or {}
                    waits = si.get("on_wait") or []
                    if len(waits) > 1 and all(
                        w.get("wait_mode") == "sem-ge-imm" for w in waits
                    ):
                        changed = True
                        eng = inst["engine"]
                        for i, w in enumerate(waits[:-1]):
                            out.append(
                                {
                                    "debug": inst.get("debug", 0),
                                    "engine": eng,
                                    "ins": [
                                        {
                                            "dtype": "int32",
                                            "kind": "imm_value",
                                            "value": 0,
                                        }
                                    ],
                                    "name": f"{inst['name']}_w{i}",
                                    "opcode": "RegisterMove",
                                    "outs": [
                                        {
                                            "dtype": "int32",
                                            "kind": "register_access",
                                            "regref": f"{eng}_zero",
                                        }
                                    ],
                                    "sync_info": {
                                        "on_update": [],
                                        "on_wait": [w],
                                    },
                                }
                            )
                        inst["sync_info"] = {
                            "on_update": si.get("on_update") or [],
                            "on_wait": [waits[-1]],
                        }
                    out.append(inst)
                blk["instructions"] = out
        if not changed:
            return bir_str
        return _json.dumps(m).encode()

    def wrapped(ant_bir_str, *args, **kwargs):
        return orig_compile(split_waits(ant_bir_str), *args, **kwargs)

    b2j.compile_bir_kernel = wrapped


def _install_ntff_hook():
    """Recreate the missing antenv.axon_hooks glue so trace=True can profile."""
    if "antenv.axon_hooks" in sys.modules:
        return
    mod = types.ModuleType("antenv.axon_hooks")
    mod._hook = None
    mod.set_axon_ntff_profile_hook = lambda h: setattr(mod, "_hook", h)
    mod.get_axon_ntff_profile_hook = lambda: mod._hook
    sys.modules["antenv.axon_hooks"] = mod
    try:
        import antenv

        antenv.axon_hooks = mod
        if "/root/.axon_site" not in sys.path:
            sys.path.insert(0, "/root/.axon_site")
        from trn_agent_boot.trn_boot import _ntff_profile_via_ctypes

        mod._hook = _ntff_profile_via_ctypes("/opt/axon/libaxon_pjrt.so")
        import concourse.bass_utils as bu

        bu.upload_artifacts = lambda tmpdir: tmpdir
    except Exception:
        pass


def build_nc():
    _patch_tile_drain()
    nc = bass.Bass()

    xq = nc.dram_tensor("xq", [128, NK, S], BF16, kind="ExternalInput")
    xk = nc.dram_tensor("xk", [128, NK, S], BF16, kind="ExternalInput")
    xv = nc.dram_tensor("xv", [128, NK, S], BF16, kind="ExternalInput")
    wq = nc.dram_tensor("wq", [NG, NK, 128, 1024], BF16, kind="ExternalInput")
    wk = nc.dram_tensor("wk", [NG, NK, 128, 1024], BF16, kind="ExternalInput")
    wo = nc.dram_tensor("wo", [NG, NK, 128, 1024], BF16, kind="ExternalInput")
    wv = nc.dram_tensor("wv", [8, NK, 128, 512], BF16, kind="ExternalInput")
    bqv = nc.dram_tensor("bqv", [128, NC], F32, kind="ExternalInput")
    bkv = nc.dram_tensor("bkv", [128, NC], F32, kind="ExternalInput")
    kmv = nc.dram_tensor("kmv", [128, NC], F32, kind="ExternalInput")
    bov = nc.dram_tensor("bov", [128, NC], F32, kind="ExternalInput")
    bvv = nc.dram_tensor("bvv", [1, D], BF16, kind="ExternalInput")
    out = nc.dram_tensor("out", [NC, 128, S], F32, kind="ExternalOutput")

    Ident = mybir.ActivationFunctionType.Identity
    Exp = mybir.ActivationFunctionType.Exp

    with tile.TileContext(nc) as tc:
        from contextlib import ExitStack

        with ExitStack() as ctx:
            resid = ctx.enter_context(tc.tile_pool(name="resid", bufs=1))
            wpool = ctx.enter_context(tc.tile_pool(name="wch", bufs=10))
            outp = ctx.enter_context(tc.tile_pool(name="outp", bufs=4))
            drp = ctx.enter_context(tc.tile_pool(name="drp", bufs=2, space="DRAM"))

            # ---- resident SBUF tensors ----
            qt_sb = resid.tile([128, NC, S], BF16, tag="qt")  # Q^T  [dout, s]
            kt_sb = resid.tile([128, NC, S], BF16, tag="kt")  # K^T  [dout, t]
            v0_sb = resid.tile([128, D], BF16, tag="v0")      # V[t=0:128, c]
            v1_sb = resid.tile([128, D], BF16, tag="v1")      # V[t=128:256, c]
            at_sb = resid.tile([128, NC, S], BF16, tag="at")  # attn^T [c, s]
            bq_sb = resid.tile([128, NC], F32, tag="bq")
            bk_sb = resid.tile([128, NC], F32, tag="bk")
            km_sb = resid.tile([128, NC], F32, tag="km")
            bo_sb = resid.tile([128, NC], F32, tag="bo")
            bv_sb = resid.tile([1, D], BF16, tag="bv")
            ones1 = resid.tile([1, 128], BF16, tag="ones1")
            ones128 = resid.tile([128, 1], BF16, tag="ones128")

            nc.vector.memset(ones1[:], 1.0)
            nc.vector.memset(ones128[:], 1.0)
            nc.gpsimd.dma_start(out=bq_sb[:], in_=bqv[:])
            nc.gpsimd.dma_start(out=bk_sb[:], in_=bkv[:])
            nc.gpsimd.dma_start(out=km_sb[:], in_=kmv[:])
            nc.gpsimd.dma_start(out=bo_sb[:], in_=bov[:])
            nc.gpsimd.dma_start(out=bv_sb[:], in_=bvv[:])

            # ---- Q and K projections (weights stationary, output transposed) ----
            def proj_t(w_dram, x_sb, dst_sb, bias_sb, scale_sb, pspool):
                for g in range(NG):
                    ps = [
                        pspool.tile([128, S], F32, name="ps", tag="ps")
                        for _ in range(8)
                    ]
                    for kt in range(NK):
                        ch = wpool.tile([128, 1024], BF16, name="ch", tag="ch")
                        eng = nc.sync if kt % 2 == 0 else nc.gpsimd
                        eng.dma_start(out=ch[:], in_=w_dram[g, kt])
                        for j in range(8):
                            nc.tensor.matmul(
                                ps[j][:],
                                lhsT=ch[:, j * 128 : (j + 1) * 128],
                                rhs=x_sb[:, kt, :],
                                start=(kt == 0),
                                stop=(kt == NK - 1),
                            )
                    for j in range(8):
                        c = g * 8 + j
                        scale = scale_sb[:, c : c + 1] if scale_sb is not None else 1.0
                        nc.scalar.activation(
                            out=dst_sb[:, c, :],
                            in_=ps[j][:],
                            func=Ident,
                            bias=bias_sb[:, c : c + 1],
                            scale=scale,
                        )

            with tc.tile_pool(name="xqk", bufs=1) as xqkp:
                xq_sb = xqkp.tile([128, NK, S], BF16, tag="xq")
                nc.sync.dma_start(out=xq_sb[:], in_=xq[:])
                xk_sb = xqkp.tile([128, NK, S], BF16, tag="xk")
                nc.gpsimd.dma_start(out=xk_sb[:], in_=xk[:])
                with tc.tile_pool(name="psqk", bufs=8, space="PSUM") as psqk:
                    proj_t(wq, xq_sb, qt_sb, bq_sb, None, psqk)
                    proj_t(wk, xk_sb, kt_sb, bk_sb, km_sb, psqk)

            # ---- V projection (x stationary, natural layout [t, c]) ----
            with tc.tile_pool(name="xvp", bufs=1) as xvp:
                xv_sb = xvp.tile([128, NK, S], BF16, tag="xv")
                nc.sync.dma_start(out=xv_sb[:], in_=xv[:])
                with tc.tile_pool(name="psv", bufs=4, space="PSUM") as psv:
                    for d in range(8):
                        pv0 = psv.tile([128, 512], F32, name="pv", tag="pv")
                        pv1 = psv.tile([128, 512], F32, name="pv", tag="pv")
                        sl = slice(d * 512, (d + 1) * 512)
                        # init PSUM with broadcast bias via K=1 matmul
                        nc.tensor.matmul(
                            pv0[:], lhsT=ones1[:], rhs=bv_sb[:, sl],
                            start=True, stop=False,
                        )
                        nc.tensor.matmul(
                            pv1[:], lhsT=ones1[:], rhs=bv_sb[:, sl],
                            start=True, stop=False,
                        )
                        for kt in range(NK):
                            ch = wpool.tile([128, 512], BF16, name="ch", tag="ch")
                            eng = nc.sync if kt % 2 == 0 else nc.gpsimd
                            eng.dma_start(out=ch[:], in_=wv[d, kt])
                            last = kt == NK - 1
                            nc.tensor.matmul(
                                pv0[:],
                                lhsT=xv_sb[:, kt, 0:128],
                                rhs=ch[:],
                                start=False,
                                stop=last,
                            )
                            nc.tensor.matmul(
                                pv1[:],
                                lhsT=xv_sb[:, kt, 128:256],
                                rhs=ch[:],
                                start=False,
                                stop=last,
                            )
                        nc.scalar.activation(out=v0_sb[:, sl], in_=pv0[:], func=Ident)
                        nc.scalar.activation(out=v1_sb[:, sl], in_=pv1[:], func=Ident)

            # ---- attention ----
            # phase A: per head scores^T, exp, and column sums; keep exp in SBUF
            with tc.tile_pool(name="attb", bufs=1) as attb:
                et_sb = attb.tile([128, NC, S], BF16, tag="et")   # exp(S^T)
                rinv = attb.tile([1, H, S], F32, tag="ri")        # 1/colsums
                rbc_sb = attb.tile([128, H, S], F32, tag="rbc")   # bcast recip
                with (
                    tc.tile_pool(name="psa", bufs=6, space="PSUM") as psa,
                    tc.tile_pool(name="psr", bufs=2, space="PSUM") as psr,
                ):
                    for h in range(H):
                        c0 = 2 * h
                        for tt in range(2):
                            pss = psa.tile([128, S], F32, name="pa", tag="pa")
                            tsl = slice(tt * 128, (tt + 1) * 128)
                            for j in range(2):
                                nc.tensor.matmul(
                                    pss[:],
                                    lhsT=kt_sb[:, c0 + j, tsl],
                                    rhs=qt_sb[:, c0 + j, :],
                                    start=(j == 0),
                                    stop=(j == 1),
                                )
                            nc.scalar.activation(
                                out=et_sb[:, c0 + tt, :], in_=pss[:], func=Exp
                            )
                        # column sums of exp (over t = partitions) via matmul
                        prs = psr.tile([1, S], F32, name="pr", tag="pr")
                        nc.tensor.matmul(
                            prs[:], lhsT=ones128[:], rhs=et_sb[:, c0, :],
                            start=True, stop=False,
                        )
                        nc.tensor.matmul(
                            prs[:], lhsT=ones128[:], rhs=et_sb[:, c0 + 1, :],
                            start=False, stop=True,
                        )
                        nc.vector.reciprocal(rinv[:, h, :], prs[:])
                    # one DRAM bounce to broadcast the reciprocals across
                    # partitions (DMA from DRAM may replicate, SBUF may not)
                    rdr = drp.tile([1, H, S], F32, name="rdr", tag="rdr")
                    nc.gpsimd.dma_start(out=rdr[:], in_=rinv[:])
                    rdr_b = bass.AP(
                        tensor=rdr.tensor,
                        offset=rdr.offset,
                        ap=[[0, 128], [S, H], [1, S]],
                    )
                    nc.gpsimd.dma_start(out=rbc_sb[:], in_=rdr_b)

                # phase B: attention @ V, normalized on PSUM->SBUF copy
                with tc.tile_pool(name="psc", bufs=6, space="PSUM") as psc:
                    for h in range(H):
                        c0 = 2 * h
                        for j in range(2):
                            csl = slice(h * HD + j * 128, h * HD + (j + 1) * 128)
                            pu = psc.tile([128, S], F32, name="pc", tag="pc")
                            nc.tensor.matmul(
                                pu[:], lhsT=v0_sb[:, csl], rhs=et_sb[:, c0, :],
                                start=True, stop=False,
                            )
                            nc.tensor.matmul(
                                pu[:], lhsT=v1_sb[:, csl], rhs=et_sb[:, c0 + 1, :],
                                start=False, stop=True,
                            )
                            nc.vector.tensor_mul(
                                at_sb[:, c0 + j, :], pu[:], rbc_sb[:, h, :]
                            )

            # ---- output projection ----
            with tc.tile_pool(name="pso", bufs=8, space="PSUM") as pso:
                for g in range(NG):
                    ps = [
                        pso.tile([128, S], F32, name="po", tag="po") for _ in range(8)
                    ]
                    for ct in range(NC):
                        ch = wpool.tile([128, 1024], BF16, name="ch", tag="ch")
                        eng = nc.sync if ct % 2 == 0 else nc.gpsimd
                        eng.dma_start(out=ch[:], in_=wo[g, ct])
                        for j in range(8):
                            nc.tensor.matmul(
                                ps[j][:],
                                lhsT=ch[:, j * 128 : (j + 1) * 128],
                                rhs=at_sb[:, ct, :],
                                start=(ct == 0),
                                stop=(ct == NC - 1),
                            )
                    for j in range(8):
                        c = g * 8 + j
                        ot = outp.tile([128, S], F32, tag="ot")
                        nc.scalar.activation(
                            out=ot[:],
                            in_=ps[j][:],
                            func=Ident,
                            bias=bo_sb[:, c : c + 1],
                        )
                        nc.sync.dma_start(out=out[c], in_=ot[:])

    return nc


_cached = {}


def _get_nc():
    if "nc" not in _cached:
        _cached["nc"] = build_nc()
    return _cached["nc"]


def _prep_shared(Wq, bq, Wk, bk, Wv, bv, Wo, bo):
    """Host-side weight reorganization (shared across cores)."""
    def chunks1024(W, scale):
        # W.T with dout split into 4 groups of 1024: [4, 32, 128, 1024]
        wt = (W.T * scale).astype(NPBF16)  # [k, dout]
        return np.ascontiguousarray(
            wt.reshape(NK, 128, NG, 1024).transpose(2, 0, 1, 3)
        )

    wq_c = chunks1024(Wq, 1.0 / 16.0)
    wk_c = chunks1024(Wk, 1.0)
    wo_c = chunks1024(Wo, 1.0 / 64.0)
    # Wv.T with dout split into 8 chunks of 512: [8, 32, 128, 512]
    wv_t = Wv.T.astype(NPBF16)
    wv_c = np.ascontiguousarray(wv_t.reshape(NK, 128, 8, 512).transpose(2, 0, 1, 3))

    bqv = np.ascontiguousarray((bq / 16.0).astype(np.float32).reshape(NC, 128).T)
    bov = np.ascontiguousarray(bo.astype(np.float32).reshape(NC, 128).T)
    bvv = np.ascontiguousarray(bv.astype(NPBF16).reshape(1, D))
    return wq_c, wk_c, wv_c, wo_c, bqv, bov, bvv


def build_in_maps(q, k, v, mask, Wq, bq, Wk, bk, Wv, bv, Wo, bo):
    q = np.asarray(q, dtype=np.float32)
    k = np.asarray(k, dtype=np.float32)
    v = np.asarray(v, dtype=np.float32)
    mask = np.asarray(mask)
    Wq, bq = np.asarray(Wq, np.float32), np.asarray(bq, np.float32)
    Wk, bk = np.asarray(Wk, np.float32), np.asarray(bk, np.float32)
    Wv, bv = np.asarray(Wv, np.float32), np.asarray(bv, np.float32)
    Wo, bo = np.asarray(Wo, np.float32), np.asarray(bo, np.float32)

    wq_c, wk_c, wv_c, wo_c, bqv, bov, bvv = _prep_shared(
        Wq, bq, Wk, bk, Wv, bv, Wo, bo
    )

    in_maps = []
    for b in range(B):
        m = (mask[b] != 0).astype(np.float32)  # [256]
        mfull = np.tile(m, H)                  # [4096] mask per channel
        bkv = np.ascontiguousarray((bk * mfull).reshape(NC, 128).T.astype(np.float32))
        kmv = np.ascontiguousarray(mfull.reshape(NC, 128).T.astype(np.float32))

        def xt(x):
            # [128 partition, NK k-tile, S] with 16KB contiguous per partition
            t = x[b].T.reshape(NK, 128, S).swapaxes(0, 1)
            return np.ascontiguousarray(t).astype(NPBF16)

        in_maps.append(
            dict(
                xq=xt(q), xk=xt(k), xv=xt(v),
                wq=wq_c, wk=wk_c, wv=wv_c, wo=wo_c,
                bqv=bqv, bkv=bkv, kmv=kmv, bov=bov, bvv=bvv,
            )
        )
    return in_maps


def unshard(results):
    outs = []
    for b in range(B):
        ot = results[b]["out"]  # [32, 128, 256]
        outs.append(ot.reshape(D, S).T)  # [256, 4096]
    return np.ascontiguousarray(np.stack(outs)).astype(np.float32)


def kernel(q, k, v, mask, Wq, bq, Wk, bk, Wv, bv, Wo, bo):
    _install_ntff_hook()
    _patch_bir_wait_split()
    nc = _get_nc()
    in_maps = build_in_maps(q, k, v, mask, Wq, bq, Wk, bk, Wv, bv, Wo, bo)

    from concourse.bass_utils import run_bass_kernel_spmd

    res = run_bass_kernel_spmd(nc, in_maps, list(range(N_CORES)))
    return unshard(res.results)
